# revision 3
# baseline (speedup 1.0000x reference)
"""Distributed multi-head attention kernel for 8 TRN2 NeuronCores.

Module: B=2, N=2048, D_MODEL=1024, H=16, D_HEAD=64 attention with
arbitrary rotary embedding, key-side boolean masking, softmax, and
output projection.

Sharding: head-parallel attention (2 heads per core, both batches),
then one AllToAll (~1 MB/core, bf16) to switch to row-parallel for the
output projection. Each core returns a [512, 1024] row block.

Key design points:
 - All matmuls bf16 with fp32 PSUM accumulation. ~5e-3 rel err.
 - qT/kT produced in [chan, row] layout so scores come out transposed
   [keys, qrows] with keys on partitions.
 - Rotary via host-rotated weight copies: rot2(x@W) == x@Wr.
 - Key mask folded into the softmax exp as a per-partition bias.
 - v computed in [chan, row] layout (one N=512 matmul per ktile) and
   PE-transposed into the [key, chan] AV layout; softmax denominator
   from a ones-column in v_aug (lhsT = [v | 1], M=65).
 - Phase 2 runs per (keytile, head): score tile [128,1024] from a
   bufs=2 PSUM pool so exp(kt,h) on ScalarE overlaps scores(kt,h+1)
   on PE. PSUM: 2x2 (sc) + 2x2 (o accum) = 8 banks.
 - Normalization happens after the AllToAll; denominators travel in
   the same buffer (shard layout [hA 64 | denA | hB 64 | denB]).
   Per-head reciprocals are spread across partitions with selector
   matmuls (not gpsimd partition_broadcast).
 - One start=True per PSUM bank per accumulation chain.
 - DMAs spread across both HWDGE queues (SP + ACT).
"""
import os
import warnings

warnings.filterwarnings("ignore")
import numpy as np
import ml_dtypes

from concourse import bacc, tile, mybir, bass_utils

B, N, DM, H, DH = 2, 2048, 1024, 16, 64
R = B * N
NCORES = 8
HPC = 2
CPC = HPC * DH       # 128 chans per core
KT = 8               # contraction tiles over d_model
RB = 8               # row blocks of 512 over R
NKEYT = 16           # key tiles of 128 over N
ROWS_PER_CORE = R // NCORES  # 512
QHS = 1024           # qrows per phase-2 inner pass

F32 = mybir.dt.float32
BF16 = mybir.dt.bfloat16

SHARD_ROWS = CPC + HPC  # 130: [hA 64 | denA 1 | hB 64 | denB 1]
VAUGW = 2 * (DH + 1)    # 130 cols per key tile: [vA | 1 | vB | 1]

LAST_EXEC_TIME_NS = None
LAST_TRACE_DIR = None


def _install_trace_shim():
    import sys
    import types
    import ctypes
    import contextlib

    if "antenv.axon_hooks" in sys.modules:
        return
    so_path = "/opt/axon/libaxon_pjrt.so"
    hook = None
    if os.path.exists(so_path):
        lib = ctypes.CDLL(so_path)
        if hasattr(lib, "axon_start_nrt_profile"):
            lib.axon_start_nrt_profile.argtypes = [
                ctypes.POINTER(ctypes.c_int64), ctypes.c_size_t]
            lib.axon_start_nrt_profile.restype = ctypes.c_int64
            lib.axon_stop_nrt_profile.argtypes = [ctypes.c_char_p]
            lib.axon_stop_nrt_profile.restype = ctypes.c_int64

            @contextlib.contextmanager
            def _hook(output_dir, device_ids):
                import jax
                jax.devices()
                if device_ids:
                    ids = (ctypes.c_int64 * len(device_ids))(*device_ids)
                    rc = lib.axon_start_nrt_profile(ids, len(device_ids))
                else:
                    rc = lib.axon_start_nrt_profile(None, 0)
                if rc != 0:
                    raise RuntimeError(f"axon_start_nrt_profile rc={rc}")
                try:
                    yield
                finally:
                    n = lib.axon_stop_nrt_profile(str(output_dir).encode())
                    print(f"[trace] {n} profile file(s) -> {output_dir}")

            hook = _hook

    mod = types.ModuleType("antenv.axon_hooks")
    mod.get_axon_ntff_profile_hook = lambda: hook
    mod.set_axon_ntff_profile_hook = lambda h: None
    sys.modules["antenv.axon_hooks"] = mod
    bass_utils.upload_artifacts = lambda tmpdir: tmpdir


def _rot_cols(w):
    wr = np.empty_like(w)
    wr[:, 0::2] = -w[:, 1::2]
    wr[:, 1::2] = w[:, 0::2]
    return wr


def build(dbg=False):
    nc = bacc.Bacc("TRN2", target_bir_lowering=False, debug=False,
                   num_devices=NCORES)

    xt_d = nc.dram_tensor("xt", [DM, R], BF16, kind="ExternalInput")
    wq_d = nc.dram_tensor("wq", [DM, CPC], BF16, kind="ExternalInput")
    wqr_d = nc.dram_tensor("wqr", [DM, CPC], BF16, kind="ExternalInput")
    wk_d = nc.dram_tensor("wk", [DM, CPC], BF16, kind="ExternalInput")
    wkr_d = nc.dram_tensor("wkr", [DM, CPC], BF16, kind="ExternalInput")
    wv_d = nc.dram_tensor("wv", [DM, CPC], BF16, kind="ExternalInput")
    wout_d = nc.dram_tensor("wout", [DM, DM], BF16, kind="ExternalInput")
    boutb_d = nc.dram_tensor("boutb", [128, DM], F32, kind="ExternalInput")
    cost_d = nc.dram_tensor("cost", [CPC, N], BF16, kind="ExternalInput")
    sint_d = nc.dram_tensor("sint", [CPC, N], BF16, kind="ExternalInput")
    maskb_d = nc.dram_tensor("maskb", [128, R // 128], F32, kind="ExternalInput")
    ident_d = nc.dram_tensor("ident", [128, 128], BF16, kind="ExternalInput")
    selk_d = nc.dram_tensor("selk", [16, KT * 128], BF16, kind="ExternalInput")
    selb_d = nc.dram_tensor("selb", [128, 1], F32, kind="ExternalInput")
    selbi_d = nc.dram_tensor("selbi", [128, 1], F32, kind="ExternalInput")

    out_d = nc.dram_tensor("out", [ROWS_PER_CORE, DM], F32, kind="ExternalOutput")

    a2a_in = [nc.dram_tensor(f"a2a_in{b}", [NCORES * SHARD_ROWS, ROWS_PER_CORE],
                             BF16) for b in range(B)]
    a2a_out = [nc.dram_tensor(f"a2a_out{b}", [NCORES * SHARD_ROWS, ROWS_PER_CORE],
                              BF16) for b in range(B)]

    with tile.TileContext(nc) as tc:
        with tc.tile_pool(name="persist", bufs=1) as pp:
            wq_sb = pp.tile([128, KT, CPC], BF16, tag="wq")
            wqr_sb = pp.tile([128, KT, CPC], BF16, tag="wqr")
            wk_sb = pp.tile([128, KT, CPC], BF16, tag="wk")
            wkr_sb = pp.tile([128, KT, CPC], BF16, tag="wkr")
            wv_sb = pp.tile([128, KT, CPC], BF16, tag="wv")
            cost_sb = pp.tile([CPC, N], BF16, tag="cost")
            sint_sb = pp.tile([CPC, N], BF16, tag="sint")
            maskb_sb = pp.tile([128, R // 128], F32, tag="maskb")
            boutb_sb = pp.tile([128, DM], F32, tag="boutb")
            ident_sb = pp.tile([128, 128], BF16, tag="ident")
            qt_sb = pp.tile([CPC, R], BF16, tag="qt")
            kt_sb = pp.tile([CPC, R], BF16, tag="kt")
            # [key-part, keytile, (vA | 1 | vB | 1)]
            va_sb = pp.tile([128, (R // 128) * VAUGW], BF16, tag="vaug")
            wo_sb = pp.tile([128, KT, DM], BF16, tag="wo")

            def ktview(d):
                return d.ap().rearrange("(k p) n -> p k n", p=128)

            xt_view = xt_d.ap().rearrange("(k p) n -> p k n", p=128)

            # first xt block + weights first so matmuls start early
            xt_sb0 = pp.tile([128, KT, 512], BF16, tag="xt0")
            for kt in range(KT):
                eng = nc.sync if kt % 2 == 0 else nc.scalar
                eng.dma_start(xt_sb0[:, kt, :], xt_view[:, kt, 0:512])
            nc.sync.dma_start(wq_sb[:], ktview(wq_d))
            nc.scalar.dma_start(wqr_sb[:], ktview(wqr_d))
            nc.sync.dma_start(wk_sb[:], ktview(wk_d))
            nc.scalar.dma_start(wkr_sb[:], ktview(wkr_d))
            nc.sync.dma_start(wv_sb[:], ktview(wv_d))
            nc.sync.dma_start(ident_sb[:], ident_d[:, :])
            # pre-load the ACT Exp table during the initial DMA wait
            warm_sb = pp.tile([1, 2], F32, tag="warm")
            nc.vector.memset(warm_sb[:], 0.0)
            nc.scalar.activation(warm_sb[0:1, 1:2], warm_sb[0:1, 0:1],
                                 mybir.ActivationFunctionType.Exp)
            nc.scalar.dma_start(cost_sb[:], cost_d[:, :])
            nc.scalar.dma_start(sint_sb[:], sint_d[:, :])
            nc.scalar.dma_start(maskb_sb[:], maskb_d[:, :])
            # ones columns of v_aug (cols 64 and 129 of each keytile slot)
            va_view = va_sb[:].rearrange("p (t w) -> p t w", w=VAUGW)
            nc.vector.memset(va_view[:, :, DH], 1.0)
            nc.vector.memset(va_view[:, :, DH + 1 + DH], 1.0)

            selk_sb = pp.tile([16, KT, 128], BF16, tag="selk")
            selb_sb = pp.tile([128, 1], F32, tag="selb")
            selbi_sb = pp.tile([128, 1], F32, tag="selbi")

            # ---- Phase 1: projections + rotary + v transpose ----
            with tc.tile_pool(name="p1", bufs=2) as p1, \
                 tc.tile_pool(name="p1v", bufs=2) as p1v, \
                 tc.tile_pool(name="ps1", bufs=1, space="PSUM") as ps1, \
                 tc.tile_pool(name="ps_tp", bufs=2, space="PSUM") as ps_tp:
                for rb in range(RB):
                    c0 = rb * 512
                    if rb == 0:
                        xt_sb = xt_sb0
                    else:
                        xt_sb = p1.tile([128, KT, 512], BF16, tag="xt")
                        if rb == 4:
                            eng = nc.gpsimd
                        elif rb % 2 == 1:
                            eng = nc.sync
                        else:
                            eng = nc.scalar
                        eng.dma_start(xt_sb[:], xt_view[:, :, c0:c0 + 512])

                    q_ps = ps1.tile([128, 512], F32, tag="q")
                    qr_ps = ps1.tile([128, 512], F32, tag="qr")
                    k_ps = ps1.tile([128, 512], F32, tag="k")
                    kr_ps = ps1.tile([128, 512], F32, tag="kr")
                    v_ps = ps1.tile([128, 512], F32, tag="v")
                    for kt in range(KT):
                        st, sp = kt == 0, kt == KT - 1
                        for ps_t, w_t in [(q_ps, wq_sb), (qr_ps, wqr_sb),
                                          (k_ps, wk_sb), (kr_ps, wkr_sb),
                                          (v_ps, wv_sb)]:
                            nc.tensor.matmul(ps_t[:], w_t[:, kt, :],
                                             xt_sb[:, kt, :], start=st, stop=sp)

                    cc = c0 % N
                    tmp = p1.tile([128, 512], BF16, tag="rottmp")
                    for dst, a_ps, b_ps in [(qt_sb, q_ps, qr_ps),
                                            (kt_sb, k_ps, kr_ps)]:
                        dv = dst[:, c0:c0 + 512]
                        nc.vector.tensor_mul(dv, a_ps[:], cost_sb[:, cc:cc + 512])
                        nc.vector.tensor_mul(tmp[:], b_ps[:], sint_sb[:, cc:cc + 512])
                        nc.vector.tensor_add(dv, dv, tmp[:])

                    # v: [chan, row] f32 -> bf16 -> PE transpose -> va slots
                    v_sb = p1v.tile([128, 512], BF16, tag="vsb")
                    nc.vector.tensor_copy(v_sb[:], v_ps[:])
                    for j in range(4):
                        t = rb * 4 + j
                        tp_ps = ps_tp.tile([128, 128], BF16, tag="tp")
                        nc.tensor.transpose(tp_ps[:], v_sb[:, j * 128:(j + 1) * 128],
                                            ident_sb[:])
                        nc.vector.tensor_copy(va_view[:, t, 0:DH],
                                              tp_ps[:, 0:DH])
                        nc.vector.tensor_copy(va_view[:, t, DH + 1:DH + 1 + DH],
                                              tp_ps[:, DH:CPC])

                # keep PE busy across the phase transition
                brid_ps = ps1.tile([128, 512], F32, tag="brid")
                for i in range(12):
                    nc.tensor.matmul(brid_ps[:], wq_sb[:, i % KT, :],
                                     xt_sb0[:, i % KT, :],
                                     start=(i == 0), stop=(i == 11))

            # phase-3 constants load behind phase-1 traffic
            nc.scalar.dma_start(wo_sb[:], wout_d.ap().rearrange(
                "(k p) n -> p k n", p=128))
            nc.sync.dma_start(boutb_sb[:], boutb_d[:, :])
            nc.sync.dma_start(selk_sb[:],
                              selk_d.ap().rearrange("h (k p) -> h k p", p=128))
            nc.sync.dma_start(selb_sb[:], selb_d[:, :])
            nc.sync.dma_start(selbi_sb[:], selbi_d[:, :])
            # zero the shard halves each batch's A2A never writes
            zt = pp.tile([128, 512], BF16, tag="zt")
            nc.vector.memset(zt[:], 0.0)
            for b in range(B):
                for j in range(NCORES):
                    if (j // 4) != b:
                        r0 = j * SHARD_ROWS
                        nc.sync.dma_start(a2a_in[b][r0:r0 + 128, :], zt[:])
                        nc.sync.dma_start(a2a_in[b][r0 + 128:r0 + SHARD_ROWS, :],
                                          zt[0:2, :])

            # ---- Phase 2: attention, per (b, q-half, keytile, head) ----
            with tc.tile_pool(name="p2", bufs=2) as p2, \
                 tc.tile_pool(name="ps_sc", bufs=2, space="PSUM") as ps_sc, \
                 tc.tile_pool(name="ps_o", bufs=1, space="PSUM") as ps_o:
                for b in range(B):
                    for qh in range(N // QHS):
                        qbase = b * N + qh * QHS
                        o_ps = [ps_o.tile([DH + 1, QHS], F32, tag=f"outp{h}",
                                          name=f"ops{h}") for h in range(HPC)]
                        for kt in range(NKEYT):
                            g = b * NKEYT + kt
                            krow = b * N + kt * 128
                            for h in range(HPC):
                                ho = h * DH
                                sc = ps_sc.tile([128, QHS], F32, tag="sc",
                                                name=f"sc{h}")
                                for qq in range(QHS // 512):
                                    nc.tensor.matmul(
                                        sc[:, qq * 512:(qq + 1) * 512],
                                        kt_sb[ho:ho + DH, krow:krow + 128],
                                        qt_sb[ho:ho + DH,
                                              qbase + qq * 512:qbase + (qq + 1) * 512],
                                        start=True, stop=True)
                                pt = p2.tile([128, QHS], BF16, tag=f"p{h}",
                                             name=f"pt{h}")
                                nc.scalar.activation(
                                    pt[:], sc[:],
                                    mybir.ActivationFunctionType.Exp,
                                    bias=maskb_sb[:, g:g + 1],
                                    scale=float(DH ** -0.5))
                                va_l = va_sb[:, g * VAUGW + h * (DH + 1):
                                             g * VAUGW + (h + 1) * (DH + 1)]
                                for qq in range(QHS // 512):
                                    nc.tensor.matmul(
                                        o_ps[h][:, qq * 512:(qq + 1) * 512],
                                        va_l,
                                        pt[:, qq * 512:(qq + 1) * 512],
                                        start=(kt == 0), stop=(kt == NKEYT - 1))

                        # tail: one bf16 copy + two [65, 512] DMAs per head
                        for h in range(HPC):
                            onb = p2.tile([DH + 1, QHS], BF16, tag=f"onb{h}",
                                          name=f"onb{h}")
                            nc.vector.tensor_copy(onb[:], o_ps[h][:])
                            for u in range(QHS // 512):
                                j = b * 4 + qh * (QHS // 512) + u
                                r0 = j * SHARD_ROWS + h * (DH + 1)
                                nc.sync.dma_start(
                                    a2a_in[b][r0: r0 + DH + 1, :],
                                    onb[:, u * 512:(u + 1) * 512])
                    if qh == N // QHS - 1:
                        nc.gpsimd.collective_compute(
                            "AllToAll", mybir.AluOpType.bypass,
                            replica_groups=[list(range(NCORES))],
                            ins=[a2a_in[b].ap().opt()],
                            outs=[a2a_out[b].ap().opt()])

            # ---- Phase 3: blend, normalize (selector-MM broadcast), project
            with tc.tile_pool(name="p3", bufs=1) as p3, \
                 tc.tile_pool(name="p3b", bufs=2) as p3b, \
                 tc.tile_pool(name="ps3", bufs=2, space="PSUM") as ps3, \
                 tc.tile_pool(name="ps_gr", bufs=2, space="PSUM") as ps_gr:
                ob = []
                dn = []
                for b in range(B):
                    o_t = p3.tile([128, KT, 512], BF16, tag=f"oallb{b}",
                                  name=f"oallb{b}")
                    d_t = p3.tile([2 * NCORES, 512], BF16, tag=f"denb{b}",
                                  name=f"denb{b}")
                    av = a2a_out[b].ap().rearrange("(j q) n -> q j n",
                                                   q=SHARD_ROWS)
                    nc.sync.dma_start(d_t[0:NCORES, :], av[DH:DH + 1, :, :])
                    nc.sync.dma_start(d_t[NCORES:2 * NCORES, :],
                                      av[CPC + 1:CPC + 2, :, :])
                    nc.sync.dma_start(o_t[0:DH, :, :], av[0:DH, :, :])
                    nc.scalar.dma_start(o_t[DH:CPC, :, :], av[DH + 1:CPC + 1, :, :])
                    ob.append(o_t)
                    dn.append(d_t)
                # blend mine = b0*sel + b1*(1-sel)
                oall_sb = p3.tile([128, KT, 512], BF16, tag="oall")
                t1_sb = p3.tile([128, KT, 512], BF16, tag="t1")
                nc.vector.tensor_scalar_mul(oall_sb[:], ob[0][:], selb_sb[:])
                nc.vector.tensor_scalar_mul(t1_sb[:], ob[1][:], selbi_sb[:])
                nc.vector.tensor_add(oall_sb[:], oall_sb[:], t1_sb[:])
                den_sb = p3.tile([2 * NCORES, 512], F32, tag="den")
                dt1_sb = p3.tile([2 * NCORES, 512], F32, tag="dt1")
                nc.vector.tensor_scalar_mul(den_sb[:], dn[0][:],
                                            selb_sb[0:2 * NCORES, :])
                nc.vector.tensor_scalar_mul(dt1_sb[:], dn[1][:],
                                            selbi_sb[0:2 * NCORES, :])
                nc.vector.tensor_add(den_sb[:], den_sb[:], dt1_sb[:])
                # den rows are [hA of peers 0-7 | hB of peers 0-7] = heads
                # interleaved: head of chan c (within peer j's 128 chans) is
                # row (c//64)*8 + j. recip then spread across partitions via
                # selector matmuls.
                recip_sb = p3.tile([2 * NCORES, 512], F32, tag="recip")
                nc.vector.reciprocal_approx_fast(recip_sb[:], den_sb[:])
                recb_sb = p3.tile([2 * NCORES, 512], BF16, tag="recb")
                nc.vector.tensor_copy(recb_sb[:], recip_sb[:])

                onorm_sb = p3.tile([128, KT, 512], BF16, tag="onorm")
                for kt in range(KT):
                    gr_ps = ps_gr.tile([128, 512], F32, tag="gr")
                    nc.tensor.matmul(gr_ps[:], selk_sb[:, kt, :], recb_sb[:],
                                     start=True, stop=True)
                    nc.vector.tensor_mul(onorm_sb[:, kt, :], oall_sb[:, kt, :],
                                         gr_ps[:])

                for rw in range(4):
                    y_ps = ps3.tile([128, DM], F32, tag="y")
                    for kt in range(KT):
                        st, sp = kt == 0, kt == KT - 1
                        for nb in range(2):
                            nc.tensor.matmul(
                                y_ps[:, nb * 512:(nb + 1) * 512],
                                onorm_sb[:, kt, rw * 128:(rw + 1) * 128],
                                wo_sb[:, kt, nb * 512:(nb + 1) * 512],
                                start=st, stop=sp)
                    y_sb = p3b.tile([128, DM], F32, tag="y_sb")
                    nc.vector.tensor_add(y_sb[:], y_ps[:], boutb_sb[:])
                    eng = nc.sync if rw % 2 == 0 else nc.scalar
                    eng.dma_start(out_d[rw * 128:(rw + 1) * 128, :], y_sb[:])

    nc.compile()
    return nc


_NC_CACHE = None


def kernel(x, mask, pos_emb, Wq, Wkv, Wout, bout):
    global LAST_EXEC_TIME_NS, LAST_TRACE_DIR, _NC_CACHE

    x = np.asarray(x, dtype=np.float32)
    mask = np.asarray(mask)
    pos_emb = np.asarray(pos_emb, dtype=np.float32)
    Wq = np.asarray(Wq, dtype=np.float32)
    Wkv = np.asarray(Wkv, dtype=np.float32)
    Wout = np.asarray(Wout, dtype=np.float32)
    bout = np.asarray(bout, dtype=np.float32)

    bf = ml_dtypes.bfloat16
    xt = np.ascontiguousarray(x.reshape(R, DM).T).astype(bf)
    wk_full = Wkv[:, :H * DH]
    wv_full = Wkv[:, H * DH:]
    cost = np.ascontiguousarray(np.tile(np.cos(pos_emb).T, (HPC, 1))).astype(bf)
    sint = np.ascontiguousarray(np.tile(np.sin(pos_emb).T, (HPC, 1))).astype(bf)
    maskb = np.ascontiguousarray(
        np.where(mask.reshape(R), 0.0, -1e5).astype(np.float32)
        .reshape(R // 128, 128).T)
    boutb = np.ascontiguousarray(
        np.broadcast_to(bout[None, :], (128, DM)).astype(np.float32))
    wqr = _rot_cols(Wq)
    wkr = _rot_cols(wk_full)
    ident = np.eye(128, dtype=bf)
    # selk[row, kt*128 + p] = 1 iff den-row `row` covers partition p of the
    # kt-th chan block. Inner chan kt*128+p belongs to peer j=kt, local
    # head p//64; its den row in d_t is (p//64)*8 + kt.
    selk = np.zeros((16, KT * 128), dtype=bf)
    for ktb in range(KT):
        for p in range(128):
            selk[(p // 64) * 8 + ktb, ktb * 128 + p] = 1.0
    in_maps = []
    for c in range(NCORES):
        cols = slice(c * CPC, (c + 1) * CPC)
        in_maps.append({
            "xt": xt,
            "wq": np.ascontiguousarray(Wq[:, cols]).astype(bf),
            "wqr": np.ascontiguousarray(wqr[:, cols]).astype(bf),
            "wk": np.ascontiguousarray(wk_full[:, cols]).astype(bf),
            "wkr": np.ascontiguousarray(wkr[:, cols]).astype(bf),
            "wv": np.ascontiguousarray(wv_full[:, cols]).astype(bf),
            "wout": Wout.astype(bf),
            "boutb": boutb,
            "cost": cost,
            "sint": sint,
            "maskb": maskb,
            "ident": ident,
            "selk": selk,
            "selb": np.full((128, 1), 1.0 if c < 4 else 0.0, dtype=np.float32),
            "selbi": np.full((128, 1), 0.0 if c < 4 else 1.0, dtype=np.float32),
        })

    dbg = bool(int(os.environ.get("BASS_KERNEL_DEBUG", "0")))
    if _NC_CACHE is None:
        _NC_CACHE = build(dbg=dbg)
    nc = _NC_CACHE

    trace = bool(int(os.environ.get("BASS_KERNEL_TRACE", "0")))
    kwargs = {}
    if trace:
        _install_trace_shim()
        tdir = os.environ.get("BASS_TRACE_DIR", "/tmp/bass_trace_out")
        os.makedirs(tdir, exist_ok=True)
        kwargs["tmpdir"] = tdir
    res = bass_utils.run_bass_kernel_spmd(
        nc, in_maps, core_ids=list(range(NCORES)), trace=trace, **kwargs)
    LAST_EXEC_TIME_NS = res.exec_time_ns
    if res.instructions_and_trace is not None:
        LAST_TRACE_DIR = res.instructions_and_trace[1]
        globals()["LAST_INSTS"] = res.instructions_and_trace[0]

    globals()["LAST_RESULTS"] = res.results
    y = np.concatenate([res.results[c]["out"] for c in range(NCORES)], axis=0)
    return y.reshape(B, N, DM)


# revision 4
# speedup vs baseline: 1.3269x; 1.3269x over previous
"""Distributed multi-head attention kernel for 8 TRN2 NeuronCores.

Module: B=2, N=2048, D_MODEL=1024, H=16, D_HEAD=64 attention with
arbitrary rotary embedding, key-side boolean masking, softmax, and
output projection.

Sharding: head-parallel attention (2 heads per core, both batches),
then one AllToAll (~1 MB/core, bf16) to switch to row-parallel for the
output projection. Each core returns a [512, 1024] row block.

Key design points:
 - All matmuls bf16 with fp32 PSUM accumulation. ~5e-3 rel err.
 - qT/kT produced in [chan, row] layout so scores come out transposed
   [keys, qrows] with keys on partitions.
 - Rotary via host-rotated weight copies: rot2(x@W) == x@Wr.
 - Key mask folded into the softmax exp as a per-partition bias.
 - v computed in [chan, row] layout (one N=512 matmul per ktile) and
   PE-transposed into the [key, chan] AV layout; softmax denominator
   from a ones-column in v_aug (lhsT = [v | 1], M=65).
 - Phase 2 runs per (keytile, head): score tile [128,1024] from a
   bufs=2 PSUM pool so exp(kt,h) on ScalarE overlaps scores(kt,h+1)
   on PE. PSUM: 2x2 (sc) + 2x2 (o accum) = 8 banks.
 - Normalization happens after the AllToAll; denominators travel in
   the same buffer (shard layout [hA 64 | denA | hB 64 | denB]).
   Per-head reciprocals are spread across partitions with selector
   matmuls (not gpsimd partition_broadcast).
 - One start=True per PSUM bank per accumulation chain.
 - DMAs spread across both HWDGE queues (SP + ACT).
"""
import os
import warnings

warnings.filterwarnings("ignore")
import numpy as np
import ml_dtypes

from concourse import bacc, tile, mybir, bass_utils

B, N, DM, H, DH = 2, 2048, 1024, 16, 64
R = B * N
NCORES = 8
HPC = 2
CPC = HPC * DH       # 128 chans per core
KT = 8               # contraction tiles over d_model
RB = 8               # row blocks of 512 over R
NKEYT = 16           # key tiles of 128 over N
ROWS_PER_CORE = R // NCORES  # 512
QHS = 1024           # qrows per phase-2 inner pass

F32 = mybir.dt.float32
BF16 = mybir.dt.bfloat16

SHARD_ROWS = CPC + HPC  # 130: [hA 64 | denA 1 | hB 64 | denB 1]
VAUGW = 2 * (DH + 1)    # 130 cols per key tile: [vA | 1 | vB | 1]

LAST_EXEC_TIME_NS = None
LAST_TRACE_DIR = None


def _install_trace_shim():
    import sys
    import types
    import ctypes
    import contextlib

    if "antenv.axon_hooks" in sys.modules:
        return
    so_path = "/opt/axon/libaxon_pjrt.so"
    hook = None
    if os.path.exists(so_path):
        lib = ctypes.CDLL(so_path)
        if hasattr(lib, "axon_start_nrt_profile"):
            lib.axon_start_nrt_profile.argtypes = [
                ctypes.POINTER(ctypes.c_int64), ctypes.c_size_t]
            lib.axon_start_nrt_profile.restype = ctypes.c_int64
            lib.axon_stop_nrt_profile.argtypes = [ctypes.c_char_p]
            lib.axon_stop_nrt_profile.restype = ctypes.c_int64

            @contextlib.contextmanager
            def _hook(output_dir, device_ids):
                import jax
                jax.devices()
                if device_ids:
                    ids = (ctypes.c_int64 * len(device_ids))(*device_ids)
                    rc = lib.axon_start_nrt_profile(ids, len(device_ids))
                else:
                    rc = lib.axon_start_nrt_profile(None, 0)
                if rc != 0:
                    raise RuntimeError(f"axon_start_nrt_profile rc={rc}")
                try:
                    yield
                finally:
                    n = lib.axon_stop_nrt_profile(str(output_dir).encode())
                    print(f"[trace] {n} profile file(s) -> {output_dir}")

            hook = _hook

    mod = types.ModuleType("antenv.axon_hooks")
    mod.get_axon_ntff_profile_hook = lambda: hook
    mod.set_axon_ntff_profile_hook = lambda h: None
    sys.modules["antenv.axon_hooks"] = mod
    bass_utils.upload_artifacts = lambda tmpdir: tmpdir


def _rot_cols(w):
    wr = np.empty_like(w)
    wr[:, 0::2] = -w[:, 1::2]
    wr[:, 1::2] = w[:, 0::2]
    return wr


def build(dbg=False):
    nc = bacc.Bacc("TRN2", target_bir_lowering=False, debug=False,
                   num_devices=NCORES)

    xt_d = nc.dram_tensor("xt", [DM, R], BF16, kind="ExternalInput")
    wq_d = nc.dram_tensor("wq", [DM, CPC], BF16, kind="ExternalInput")
    wqr_d = nc.dram_tensor("wqr", [DM, CPC], BF16, kind="ExternalInput")
    wk_d = nc.dram_tensor("wk", [DM, CPC], BF16, kind="ExternalInput")
    wkr_d = nc.dram_tensor("wkr", [DM, CPC], BF16, kind="ExternalInput")
    wv_d = nc.dram_tensor("wv", [DM, CPC], BF16, kind="ExternalInput")
    wout_d = nc.dram_tensor("wout", [DM, DM], BF16, kind="ExternalInput")
    boutb_d = nc.dram_tensor("boutb", [128, DM], F32, kind="ExternalInput")
    cost_d = nc.dram_tensor("cost", [CPC, N], BF16, kind="ExternalInput")
    sint_d = nc.dram_tensor("sint", [CPC, N], BF16, kind="ExternalInput")
    maskb_d = nc.dram_tensor("maskb", [128, R // 128], F32, kind="ExternalInput")
    ident_d = nc.dram_tensor("ident", [128, 128], BF16, kind="ExternalInput")
    selk_d = nc.dram_tensor("selk", [16, KT * 128], BF16, kind="ExternalInput")
    selb_d = nc.dram_tensor("selb", [128, 1], F32, kind="ExternalInput")
    selbi_d = nc.dram_tensor("selbi", [128, 1], F32, kind="ExternalInput")

    out_d = nc.dram_tensor("out", [ROWS_PER_CORE, DM], F32, kind="ExternalOutput")

    a2a_in = [nc.dram_tensor(f"a2a_in{b}", [NCORES * SHARD_ROWS, ROWS_PER_CORE],
                             BF16) for b in range(B)]
    a2a_out = [nc.dram_tensor(f"a2a_out{b}", [NCORES * SHARD_ROWS, ROWS_PER_CORE],
                              BF16) for b in range(B)]

    with tile.TileContext(nc) as tc:
        with tc.tile_pool(name="persist", bufs=1) as pp:
            wq_sb = pp.tile([128, KT, CPC], BF16, tag="wq")
            wqr_sb = pp.tile([128, KT, CPC], BF16, tag="wqr")
            wk_sb = pp.tile([128, KT, CPC], BF16, tag="wk")
            wkr_sb = pp.tile([128, KT, CPC], BF16, tag="wkr")
            wv_sb = pp.tile([128, KT, CPC], BF16, tag="wv")
            cost_sb = pp.tile([CPC, N], BF16, tag="cost")
            sint_sb = pp.tile([CPC, N], BF16, tag="sint")
            maskb_sb = pp.tile([128, R // 128], F32, tag="maskb")
            boutb_sb = pp.tile([128, DM], F32, tag="boutb")
            ident_sb = pp.tile([128, 128], BF16, tag="ident")
            qt_sb = pp.tile([CPC, R], BF16, tag="qt")
            kt_sb = pp.tile([CPC, R], BF16, tag="kt")
            # [key-part, keytile, (vA | 1 | vB | 1)]
            va_sb = pp.tile([128, (R // 128) * VAUGW], BF16, tag="vaug")
            wo_sb = pp.tile([128, KT, DM], BF16, tag="wo")

            def ktview(d):
                return d.ap().rearrange("(k p) n -> p k n", p=128)

            xt_view = xt_d.ap().rearrange("(k p) n -> p k n", p=128)

            # first xt block + weights first so matmuls start early
            xt_sb0 = pp.tile([128, KT, 512], BF16, tag="xt0")
            for kt in range(KT):
                eng = nc.sync if kt % 2 == 0 else nc.scalar
                eng.dma_start(xt_sb0[:, kt, :], xt_view[:, kt, 0:512])
            nc.sync.dma_start(wq_sb[:], ktview(wq_d))
            nc.scalar.dma_start(wqr_sb[:], ktview(wqr_d))
            nc.sync.dma_start(wk_sb[:], ktview(wk_d))
            nc.scalar.dma_start(wkr_sb[:], ktview(wkr_d))
            nc.sync.dma_start(wv_sb[:], ktview(wv_d))
            nc.sync.dma_start(ident_sb[:], ident_d[:, :])
            # pre-load the ACT Exp table during the initial DMA wait
            warm_sb = pp.tile([1, 2], F32, tag="warm")
            nc.vector.memset(warm_sb[:], 0.0)
            nc.scalar.activation(warm_sb[0:1, 1:2], warm_sb[0:1, 0:1],
                                 mybir.ActivationFunctionType.Exp)
            nc.scalar.dma_start(cost_sb[:], cost_d[:, :])
            nc.scalar.dma_start(sint_sb[:], sint_d[:, :])
            nc.scalar.dma_start(maskb_sb[:], maskb_d[:, :])
            # ones columns of v_aug (cols 64 and 129 of each keytile slot)
            va_view = va_sb[:].rearrange("p (t w) -> p t w", w=VAUGW)
            nc.vector.memset(va_view[:, :, DH], 1.0)
            nc.vector.memset(va_view[:, :, DH + 1 + DH], 1.0)

            selk_sb = pp.tile([16, KT, 128], BF16, tag="selk")
            selb_sb = pp.tile([128, 1], F32, tag="selb")
            selbi_sb = pp.tile([128, 1], F32, tag="selbi")

            # ---- Phase 1: projections + rotary + v transpose ----
            with tc.tile_pool(name="p1", bufs=2) as p1, \
                 tc.tile_pool(name="p1v", bufs=2) as p1v, \
                 tc.tile_pool(name="ps1", bufs=1, space="PSUM") as ps1, \
                 tc.tile_pool(name="ps_tp", bufs=2, space="PSUM") as ps_tp:
                for rb in range(RB):
                    c0 = rb * 512
                    if rb == 0:
                        xt_sb = xt_sb0
                    else:
                        xt_sb = p1.tile([128, KT, 512], BF16, tag="xt")
                        if rb == 4:
                            eng = nc.gpsimd
                        elif rb % 2 == 1:
                            eng = nc.sync
                        else:
                            eng = nc.scalar
                        eng.dma_start(xt_sb[:], xt_view[:, :, c0:c0 + 512])

                    q_ps = ps1.tile([128, 512], F32, tag="q")
                    qr_ps = ps1.tile([128, 512], F32, tag="qr")
                    k_ps = ps1.tile([128, 512], F32, tag="k")
                    kr_ps = ps1.tile([128, 512], F32, tag="kr")
                    v_ps = ps1.tile([128, 512], F32, tag="v")
                    for kt in range(KT):
                        st, sp = kt == 0, kt == KT - 1
                        for ps_t, w_t in [(q_ps, wq_sb), (qr_ps, wqr_sb),
                                          (k_ps, wk_sb), (kr_ps, wkr_sb),
                                          (v_ps, wv_sb)]:
                            nc.tensor.matmul(ps_t[:], w_t[:, kt, :],
                                             xt_sb[:, kt, :], start=st, stop=sp)

                    cc = c0 % N
                    tmp = p1.tile([128, 512], BF16, tag="rottmp")
                    for dst, a_ps, b_ps in [(qt_sb, q_ps, qr_ps),
                                            (kt_sb, k_ps, kr_ps)]:
                        dv = dst[:, c0:c0 + 512]
                        nc.vector.tensor_mul(dv, a_ps[:], cost_sb[:, cc:cc + 512])
                        nc.vector.tensor_mul(tmp[:], b_ps[:], sint_sb[:, cc:cc + 512])
                        nc.vector.tensor_add(dv, dv, tmp[:])

                    # v: [chan, row] f32 -> bf16 -> PE transpose -> va slots
                    v_sb = p1v.tile([128, 512], BF16, tag="vsb")
                    nc.vector.tensor_copy(v_sb[:], v_ps[:])
                    for j in range(4):
                        t = rb * 4 + j
                        tp_ps = ps_tp.tile([128, 128], BF16, tag="tp")
                        nc.tensor.transpose(tp_ps[:], v_sb[:, j * 128:(j + 1) * 128],
                                            ident_sb[:])
                        nc.vector.tensor_copy(va_view[:, t, 0:DH],
                                              tp_ps[:, 0:DH])
                        nc.vector.tensor_copy(va_view[:, t, DH + 1:DH + 1 + DH],
                                              tp_ps[:, DH:CPC])

                # keep PE busy across the phase transition
                brid_ps = ps1.tile([128, 512], F32, tag="brid")
                for i in range(12):
                    nc.tensor.matmul(brid_ps[:], wq_sb[:, i % KT, :],
                                     xt_sb0[:, i % KT, :],
                                     start=(i == 0), stop=(i == 11))

            # phase-3 constants load behind phase-1 traffic
            nc.scalar.dma_start(wo_sb[:], wout_d.ap().rearrange(
                "(k p) n -> p k n", p=128))
            nc.sync.dma_start(boutb_sb[:], boutb_d[:, :])
            nc.sync.dma_start(selk_sb[:],
                              selk_d.ap().rearrange("h (k p) -> h k p", p=128))
            nc.sync.dma_start(selb_sb[:], selb_d[:, :])
            nc.sync.dma_start(selbi_sb[:], selbi_d[:, :])
            # zero the shard halves each batch's A2A never writes
            zt = pp.tile([128, 512], BF16, tag="zt")
            nc.vector.memset(zt[:], 0.0)
            for b in range(B):
                for j in range(NCORES):
                    if (j // 4) != b:
                        r0 = j * SHARD_ROWS
                        nc.sync.dma_start(a2a_in[b][r0:r0 + 128, :], zt[:])
                        nc.sync.dma_start(a2a_in[b][r0 + 128:r0 + SHARD_ROWS, :],
                                          zt[0:2, :])

            # ---- Phase 2: attention, per (b, q-chunk of 512, keytile) ----
            # sc [128, 1024] packs both heads' [keys, 512 qrows] blocks side
            # by side so one exp call covers both; bufs=2 double-buffers the
            # PE-scores vs ACT-exp pipeline. PSUM: 2x2 (sc) + 2x1 (o) = 6.
            with tc.tile_pool(name="p2", bufs=2) as p2, \
                 tc.tile_pool(name="ps_sc", bufs=2, space="PSUM") as ps_sc, \
                 tc.tile_pool(name="ps_o", bufs=1, space="PSUM") as ps_o:
                QC = 512
                for b in range(B):
                    for qh in range(N // QC):
                        qbase = b * N + qh * QC
                        o_ps = [ps_o.tile([DH + 1, QC], F32, tag=f"outp{h}",
                                          name=f"ops{h}") for h in range(HPC)]
                        for kt in range(NKEYT):
                            g = b * NKEYT + kt
                            krow = b * N + kt * 128
                            sc = ps_sc.tile([128, 2 * QC], F32, tag="sc",
                                            name="sc")
                            for h in range(HPC):
                                ho = h * DH
                                nc.tensor.matmul(
                                    sc[:, h * QC:(h + 1) * QC],
                                    kt_sb[ho:ho + DH, krow:krow + 128],
                                    qt_sb[ho:ho + DH, qbase:qbase + QC],
                                    start=True, stop=True)
                            pt = p2.tile([128, 2 * QC], BF16, tag="pt",
                                         name="pt")
                            nc.scalar.activation(
                                pt[:], sc[:],
                                mybir.ActivationFunctionType.Exp,
                                bias=maskb_sb[:, g:g + 1],
                                scale=float(DH ** -0.5))
                            for h in range(HPC):
                                va_l = va_sb[:, g * VAUGW + h * (DH + 1):
                                             g * VAUGW + (h + 1) * (DH + 1)]
                                nc.tensor.matmul(
                                    o_ps[h][:], va_l, pt[:, h * QC:(h + 1) * QC],
                                    start=(kt == 0), stop=(kt == NKEYT - 1))

                        # tail: one bf16 copy + one [65, 512] DMA per head
                        j = b * 4 + qh
                        for h in range(HPC):
                            onb = p2.tile([DH + 1, QC], BF16, tag=f"onb{h}",
                                          name=f"onb{h}")
                            nc.vector.tensor_copy(onb[:], o_ps[h][:])
                            r0 = j * SHARD_ROWS + h * (DH + 1)
                            nc.sync.dma_start(a2a_in[b][r0: r0 + DH + 1, :],
                                              onb[:])
                    nc.gpsimd.collective_compute(
                        "AllToAll", mybir.AluOpType.bypass,
                        replica_groups=[list(range(NCORES))],
                        ins=[a2a_in[b].ap().opt()],
                        outs=[a2a_out[b].ap().opt()])

            # ---- Phase 3: blend, normalize (selector-MM broadcast), project
            with tc.tile_pool(name="p3", bufs=1) as p3, \
                 tc.tile_pool(name="p3b", bufs=2) as p3b, \
                 tc.tile_pool(name="ps3", bufs=2, space="PSUM") as ps3, \
                 tc.tile_pool(name="ps_gr", bufs=2, space="PSUM") as ps_gr:
                ob = []
                dn = []
                for b in range(B):
                    o_t = p3.tile([128, KT, 512], BF16, tag=f"oallb{b}",
                                  name=f"oallb{b}")
                    d_t = p3.tile([2 * NCORES, 512], BF16, tag=f"denb{b}",
                                  name=f"denb{b}")
                    av = a2a_out[b].ap().rearrange("(j q) n -> q j n",
                                                   q=SHARD_ROWS)
                    nc.sync.dma_start(d_t[0:NCORES, :], av[DH:DH + 1, :, :])
                    nc.sync.dma_start(d_t[NCORES:2 * NCORES, :],
                                      av[CPC + 1:CPC + 2, :, :])
                    nc.sync.dma_start(o_t[0:DH, :, :], av[0:DH, :, :])
                    nc.scalar.dma_start(o_t[DH:CPC, :, :], av[DH + 1:CPC + 1, :, :])
                    ob.append(o_t)
                    dn.append(d_t)
                # blend mine = b0*sel + b1*(1-sel)
                oall_sb = p3.tile([128, KT, 512], BF16, tag="oall")
                t1_sb = p3.tile([128, KT, 512], BF16, tag="t1")
                nc.vector.tensor_scalar_mul(oall_sb[:], ob[0][:], selb_sb[:])
                nc.vector.tensor_scalar_mul(t1_sb[:], ob[1][:], selbi_sb[:])
                nc.vector.tensor_add(oall_sb[:], oall_sb[:], t1_sb[:])
                den_sb = p3.tile([2 * NCORES, 512], F32, tag="den")
                dt1_sb = p3.tile([2 * NCORES, 512], F32, tag="dt1")
                nc.vector.tensor_scalar_mul(den_sb[:], dn[0][:],
                                            selb_sb[0:2 * NCORES, :])
                nc.vector.tensor_scalar_mul(dt1_sb[:], dn[1][:],
                                            selbi_sb[0:2 * NCORES, :])
                nc.vector.tensor_add(den_sb[:], den_sb[:], dt1_sb[:])
                # den rows are [hA of peers 0-7 | hB of peers 0-7] = heads
                # interleaved: head of chan c (within peer j's 128 chans) is
                # row (c//64)*8 + j. recip then spread across partitions via
                # selector matmuls.
                recip_sb = p3.tile([2 * NCORES, 512], F32, tag="recip")
                nc.vector.reciprocal_approx_fast(recip_sb[:], den_sb[:])
                recb_sb = p3.tile([2 * NCORES, 512], BF16, tag="recb")
                nc.vector.tensor_copy(recb_sb[:], recip_sb[:])

                onorm_sb = p3.tile([128, KT, 512], BF16, tag="onorm")
                for kt in range(KT):
                    gr_ps = ps_gr.tile([128, 512], F32, tag="gr")
                    nc.tensor.matmul(gr_ps[:], selk_sb[:, kt, :], recb_sb[:],
                                     start=True, stop=True)
                    nc.vector.tensor_mul(onorm_sb[:, kt, :], oall_sb[:, kt, :],
                                         gr_ps[:])

                for rw in range(4):
                    y_ps = ps3.tile([128, DM], F32, tag="y")
                    for kt in range(KT):
                        st, sp = kt == 0, kt == KT - 1
                        for nb in range(2):
                            nc.tensor.matmul(
                                y_ps[:, nb * 512:(nb + 1) * 512],
                                onorm_sb[:, kt, rw * 128:(rw + 1) * 128],
                                wo_sb[:, kt, nb * 512:(nb + 1) * 512],
                                start=st, stop=sp)
                    y_sb = p3b.tile([128, DM], F32, tag="y_sb")
                    nc.vector.tensor_add(y_sb[:], y_ps[:], boutb_sb[:])
                    eng = nc.sync if rw % 2 == 0 else nc.scalar
                    eng.dma_start(out_d[rw * 128:(rw + 1) * 128, :], y_sb[:])

    nc.compile()
    return nc


_NC_CACHE = None


def kernel(x, mask, pos_emb, Wq, Wkv, Wout, bout):
    global LAST_EXEC_TIME_NS, LAST_TRACE_DIR, _NC_CACHE

    x = np.asarray(x, dtype=np.float32)
    mask = np.asarray(mask)
    pos_emb = np.asarray(pos_emb, dtype=np.float32)
    Wq = np.asarray(Wq, dtype=np.float32)
    Wkv = np.asarray(Wkv, dtype=np.float32)
    Wout = np.asarray(Wout, dtype=np.float32)
    bout = np.asarray(bout, dtype=np.float32)

    bf = ml_dtypes.bfloat16
    xt = np.ascontiguousarray(x.reshape(R, DM).T).astype(bf)
    wk_full = Wkv[:, :H * DH]
    wv_full = Wkv[:, H * DH:]
    cost = np.ascontiguousarray(np.tile(np.cos(pos_emb).T, (HPC, 1))).astype(bf)
    sint = np.ascontiguousarray(np.tile(np.sin(pos_emb).T, (HPC, 1))).astype(bf)
    maskb = np.ascontiguousarray(
        np.where(mask.reshape(R), 0.0, -1e5).astype(np.float32)
        .reshape(R // 128, 128).T)
    boutb = np.ascontiguousarray(
        np.broadcast_to(bout[None, :], (128, DM)).astype(np.float32))
    wqr = _rot_cols(Wq)
    wkr = _rot_cols(wk_full)
    ident = np.eye(128, dtype=bf)
    # selk[row, kt*128 + p] = 1 iff den-row `row` covers partition p of the
    # kt-th chan block. Inner chan kt*128+p belongs to peer j=kt, local
    # head p//64; its den row in d_t is (p//64)*8 + kt.
    selk = np.zeros((16, KT * 128), dtype=bf)
    for ktb in range(KT):
        for p in range(128):
            selk[(p // 64) * 8 + ktb, ktb * 128 + p] = 1.0
    in_maps = []
    for c in range(NCORES):
        cols = slice(c * CPC, (c + 1) * CPC)
        in_maps.append({
            "xt": xt,
            "wq": np.ascontiguousarray(Wq[:, cols]).astype(bf),
            "wqr": np.ascontiguousarray(wqr[:, cols]).astype(bf),
            "wk": np.ascontiguousarray(wk_full[:, cols]).astype(bf),
            "wkr": np.ascontiguousarray(wkr[:, cols]).astype(bf),
            "wv": np.ascontiguousarray(wv_full[:, cols]).astype(bf),
            "wout": Wout.astype(bf),
            "boutb": boutb,
            "cost": cost,
            "sint": sint,
            "maskb": maskb,
            "ident": ident,
            "selk": selk,
            "selb": np.full((128, 1), 1.0 if c < 4 else 0.0, dtype=np.float32),
            "selbi": np.full((128, 1), 0.0 if c < 4 else 1.0, dtype=np.float32),
        })

    dbg = bool(int(os.environ.get("BASS_KERNEL_DEBUG", "0")))
    if _NC_CACHE is None:
        _NC_CACHE = build(dbg=dbg)
    nc = _NC_CACHE

    trace = bool(int(os.environ.get("BASS_KERNEL_TRACE", "0")))
    kwargs = {}
    if trace:
        _install_trace_shim()
        tdir = os.environ.get("BASS_TRACE_DIR", "/tmp/bass_trace_out")
        os.makedirs(tdir, exist_ok=True)
        kwargs["tmpdir"] = tdir
    res = bass_utils.run_bass_kernel_spmd(
        nc, in_maps, core_ids=list(range(NCORES)), trace=trace, **kwargs)
    LAST_EXEC_TIME_NS = res.exec_time_ns
    if res.instructions_and_trace is not None:
        LAST_TRACE_DIR = res.instructions_and_trace[1]
        globals()["LAST_INSTS"] = res.instructions_and_trace[0]

    globals()["LAST_RESULTS"] = res.results
    y = np.concatenate([res.results[c]["out"] for c in range(NCORES)], axis=0)
    return y.reshape(B, N, DM)


# revision 37
# speedup vs baseline: 1.4638x; 1.1031x over previous
"""Distributed multi-head attention kernel for 8 TRN2 NeuronCores.

Module: B=2, N=2048, D_MODEL=1024, H=16, D_HEAD=64 attention with
arbitrary rotary embedding, key-side boolean masking, softmax, and
output projection.

Sharding: head-parallel attention (2 heads per core, both batches),
then one AllToAll (~1 MB/core, bf16) to switch to row-parallel for the
output projection. Each core returns a [512, 1024] row block.

Key design points:
 - All matmuls bf16 with fp32 PSUM accumulation. ~5e-3 rel err.
 - qT/kT produced in [chan, row] layout so scores come out transposed
   [keys, qrows] with keys on partitions.
 - Rotary via host-rotated weight copies: rot2(x@W) == x@Wr.
 - Key mask folded into the softmax exp as a per-partition bias.
 - v computed in [chan, row] layout (one N=512 matmul per ktile) and
   PE-transposed into the [key, chan] AV layout; softmax denominator
   from a ones-column in v_aug (lhsT = [v | 1], M=65).
 - Phase 2 runs per (keytile, head): score tile [128,1024] from a
   bufs=2 PSUM pool so exp(kt,h) on ScalarE overlaps scores(kt,h+1)
   on PE. PSUM: 2x2 (sc) + 2x2 (o accum) = 8 banks.
 - Normalization happens after the AllToAll; denominators travel in
   the same buffer (shard layout [hA 64 | denA | hB 64 | denB]).
   Per-head reciprocals are spread across partitions with selector
   matmuls (not gpsimd partition_broadcast).
 - One start=True per PSUM bank per accumulation chain.
 - DMAs spread across both HWDGE queues (SP + ACT).
"""
import os
import warnings

warnings.filterwarnings("ignore")
import numpy as np
import ml_dtypes

from concourse import bacc, tile, mybir, bass_utils

B, N, DM, H, DH = 2, 2048, 1024, 16, 64
R = B * N
NCORES = 8
HPC = 2
CPC = HPC * DH       # 128 chans per core
KT = 8               # contraction tiles over d_model
RB = 8               # row blocks of 512 over R
NKEYT = 16           # key tiles of 128 over N
ROWS_PER_CORE = R // NCORES  # 512
QHS = 1024           # qrows per phase-2 inner pass

F32 = mybir.dt.float32
BF16 = mybir.dt.bfloat16
I16 = mybir.dt.int16

# Schraudolph bf16 exp constants: exp(z) ~= bitcast_bf16(int16(SCH_A*z + SCH_B))
SCH_A = 184.66496      # 128/ln2
SCH_B = 16248.6        # 127*128 - sigma (round-to-nearest convert)
SCH_SCALE = SCH_A * (DH ** -0.5)  # folds the 1/sqrt(d) score scale in
SCH_ON = bool(int(os.environ.get("BASS_SCH", "0")))

SHARD_ROWS = CPC + HPC  # 130: [hA 64 | denA 1 | hB 64 | denB 1]
VAUGW = 2 * (DH + 1)    # 130 cols per key tile: [vA | 1 | vB | 1]

LAST_EXEC_TIME_NS = None
LAST_TRACE_DIR = None


def _install_trace_shim():
    import sys
    import types
    import ctypes
    import contextlib

    if "antenv.axon_hooks" in sys.modules:
        return
    so_path = "/opt/axon/libaxon_pjrt.so"
    hook = None
    if os.path.exists(so_path):
        lib = ctypes.CDLL(so_path)
        if hasattr(lib, "axon_start_nrt_profile"):
            lib.axon_start_nrt_profile.argtypes = [
                ctypes.POINTER(ctypes.c_int64), ctypes.c_size_t]
            lib.axon_start_nrt_profile.restype = ctypes.c_int64
            lib.axon_stop_nrt_profile.argtypes = [ctypes.c_char_p]
            lib.axon_stop_nrt_profile.restype = ctypes.c_int64

            @contextlib.contextmanager
            def _hook(output_dir, device_ids):
                import jax
                jax.devices()
                if device_ids:
                    ids = (ctypes.c_int64 * len(device_ids))(*device_ids)
                    rc = lib.axon_start_nrt_profile(ids, len(device_ids))
                else:
                    rc = lib.axon_start_nrt_profile(None, 0)
                if rc != 0:
                    raise RuntimeError(f"axon_start_nrt_profile rc={rc}")
                try:
                    yield
                finally:
                    n = lib.axon_stop_nrt_profile(str(output_dir).encode())
                    print(f"[trace] {n} profile file(s) -> {output_dir}")

            hook = _hook

    mod = types.ModuleType("antenv.axon_hooks")
    mod.get_axon_ntff_profile_hook = lambda: hook
    mod.set_axon_ntff_profile_hook = lambda h: None
    sys.modules["antenv.axon_hooks"] = mod
    bass_utils.upload_artifacts = lambda tmpdir: tmpdir


def _rot_cols(w):
    wr = np.empty_like(w)
    wr[:, 0::2] = -w[:, 1::2]
    wr[:, 1::2] = w[:, 0::2]
    return wr


def build(dbg=False):
    nc = bacc.Bacc("TRN2", target_bir_lowering=False, debug=False,
                   num_devices=NCORES)

    xt_d = nc.dram_tensor("xt", [DM, R], BF16, kind="ExternalInput")
    wq_d = nc.dram_tensor("wq", [DM, CPC], BF16, kind="ExternalInput")
    wqr_d = nc.dram_tensor("wqr", [DM, CPC], BF16, kind="ExternalInput")
    wk_d = nc.dram_tensor("wk", [DM, CPC], BF16, kind="ExternalInput")
    wkr_d = nc.dram_tensor("wkr", [DM, CPC], BF16, kind="ExternalInput")
    wv_d = nc.dram_tensor("wv", [DM, CPC], BF16, kind="ExternalInput")
    wout_d = nc.dram_tensor("wout", [DM, DM], BF16, kind="ExternalInput")
    boutb_d = nc.dram_tensor("boutb", [128, DM], F32, kind="ExternalInput")
    cost_d = nc.dram_tensor("cost", [CPC, N], BF16, kind="ExternalInput")
    sint_d = nc.dram_tensor("sint", [CPC, N], BF16, kind="ExternalInput")
    maskb_d = nc.dram_tensor("maskb", [128, R // 128], F32, kind="ExternalInput")
    ident_d = nc.dram_tensor("ident", [128, 128], BF16, kind="ExternalInput")
    selk_d = nc.dram_tensor("selk", [16, KT * 128], BF16, kind="ExternalInput")
    selb_d = nc.dram_tensor("selb", [128, 1], F32, kind="ExternalInput")
    selbi_d = nc.dram_tensor("selbi", [128, 1], F32, kind="ExternalInput")

    out_d = nc.dram_tensor("out", [ROWS_PER_CORE, DM], F32, kind="ExternalOutput")

    a2a_in = [nc.dram_tensor(f"a2a_in{b}", [NCORES * SHARD_ROWS, ROWS_PER_CORE],
                             BF16) for b in range(B)]
    a2a_out = [nc.dram_tensor(f"a2a_out{b}", [NCORES * SHARD_ROWS, ROWS_PER_CORE],
                              BF16) for b in range(B)]

    with tile.TileContext(nc) as tc:
        with tc.tile_pool(name="persist", bufs=1) as pp:
            wq_sb = pp.tile([128, KT, CPC], BF16, tag="wq")
            wqr_sb = pp.tile([128, KT, CPC], BF16, tag="wqr")
            wk_sb = pp.tile([128, KT, CPC], BF16, tag="wk")
            wkr_sb = pp.tile([128, KT, CPC], BF16, tag="wkr")
            wv_sb = pp.tile([128, KT, CPC], BF16, tag="wv")
            cost_sb = pp.tile([CPC, N], BF16, tag="cost")
            sint_sb = pp.tile([CPC, N], BF16, tag="sint")
            maskb_sb = pp.tile([128, R // 128], F32, tag="maskb")
            # Schraudolph bias for the DVE exp path:
            # int16 bits = round(SCH_SCALE*score + (SCH_A*maskbias + SCH_B))
            bias2_sb = pp.tile([128, R // 128], F32, tag="bias2")
            boutb_sb = pp.tile([128, DM], F32, tag="boutb")
            ident_sb = pp.tile([128, 128], BF16, tag="ident")
            qt_sb = pp.tile([CPC, R], BF16, tag="qt")
            kt_sb = pp.tile([CPC, R], BF16, tag="kt")
            # [key-part, keytile, (vA | 1 | vB | 1)]
            va_sb = pp.tile([128, (R // 128) * VAUGW], BF16, tag="vaug")
            wo_sb = pp.tile([128, KT, DM], BF16, tag="wo")

            def ktview(d):
                return d.ap().rearrange("(k p) n -> p k n", p=128)

            xt_view = xt_d.ap().rearrange("(k p) n -> p k n", p=128)

            # first xt block + weights first so matmuls start early
            xt_sb0 = pp.tile([128, KT, 512], BF16, tag="xt0")
            for kt in range(KT):
                eng = nc.sync if kt % 2 == 0 else nc.scalar
                eng.dma_start(xt_sb0[:, kt, :], xt_view[:, kt, 0:512])
            nc.sync.dma_start(wq_sb[:], ktview(wq_d))
            nc.scalar.dma_start(wqr_sb[:], ktview(wqr_d))
            nc.sync.dma_start(wk_sb[:], ktview(wk_d))
            nc.scalar.dma_start(wkr_sb[:], ktview(wkr_d))
            nc.sync.dma_start(wv_sb[:], ktview(wv_d))
            nc.sync.dma_start(ident_sb[:], ident_d[:, :])
            # pre-load the ACT Exp table during the initial DMA wait
            warm_sb = pp.tile([1, 2], F32, tag="warm")
            nc.vector.memset(warm_sb[:], 0.0)
            nc.scalar.activation(warm_sb[0:1, 1:2], warm_sb[0:1, 0:1],
                                 mybir.ActivationFunctionType.Exp)
            nc.scalar.dma_start(cost_sb[:], cost_d[:, :])
            nc.scalar.dma_start(sint_sb[:], sint_d[:, :])
            nc.scalar.dma_start(maskb_sb[:], maskb_d[:, :])
            nc.vector.tensor_scalar(
                out=bias2_sb[:], in0=maskb_sb[:], scalar1=SCH_A,
                scalar2=SCH_B, op0=mybir.AluOpType.mult,
                op1=mybir.AluOpType.add)
            # ones columns of v_aug (cols 64 and 129 of each keytile slot)
            va_view = va_sb[:].rearrange("p (t w) -> p t w", w=VAUGW)
            nc.vector.memset(va_view[:, :, DH], 1.0)
            nc.vector.memset(va_view[:, :, DH + 1 + DH], 1.0)

            selk_sb = pp.tile([16, KT, 128], BF16, tag="selk")
            selb_sb = pp.tile([128, 1], F32, tag="selb")
            selbi_sb = pp.tile([128, 1], F32, tag="selbi")

            # ---- Phase 1: projections + rotary + v transpose ----
            with tc.tile_pool(name="p1", bufs=2) as p1, \
                 tc.tile_pool(name="p1v", bufs=2) as p1v, \
                 tc.tile_pool(name="ps1", bufs=1, space="PSUM") as ps1, \
                 tc.tile_pool(name="ps_tp", bufs=2, space="PSUM") as ps_tp:
                for rb in range(RB):
                    c0 = rb * 512
                    if rb == 0:
                        xt_sb = xt_sb0
                    else:
                        xt_sb = p1.tile([128, KT, 512], BF16, tag="xt")
                        if rb == 4:
                            eng = nc.gpsimd
                        elif rb % 2 == 1:
                            eng = nc.sync
                        else:
                            eng = nc.scalar
                        eng.dma_start(xt_sb[:], xt_view[:, :, c0:c0 + 512])

                    q_ps = ps1.tile([128, 512], F32, tag="q")
                    qr_ps = ps1.tile([128, 512], F32, tag="qr")
                    k_ps = ps1.tile([128, 512], F32, tag="k")
                    kr_ps = ps1.tile([128, 512], F32, tag="kr")
                    v_ps = ps1.tile([128, 512], F32, tag="v")
                    for kt in range(KT):
                        st, sp = kt == 0, kt == KT - 1
                        for ps_t, w_t in [(q_ps, wq_sb), (qr_ps, wqr_sb),
                                          (k_ps, wk_sb), (kr_ps, wkr_sb),
                                          (v_ps, wv_sb)]:
                            nc.tensor.matmul(ps_t[:], w_t[:, kt, :],
                                             xt_sb[:, kt, :], start=st, stop=sp)

                    cc = c0 % N
                    tmp = p1.tile([128, 512], BF16, tag="rottmp")
                    for dst, a_ps, b_ps in [(qt_sb, q_ps, qr_ps),
                                            (kt_sb, k_ps, kr_ps)]:
                        dv = dst[:, c0:c0 + 512]
                        nc.vector.tensor_mul(dv, a_ps[:], cost_sb[:, cc:cc + 512])
                        nc.vector.tensor_mul(tmp[:], b_ps[:], sint_sb[:, cc:cc + 512])
                        nc.vector.tensor_add(dv, dv, tmp[:])

                    # v: [chan, row] f32 -> bf16 -> PE transpose -> va slots
                    v_sb = p1v.tile([128, 512], BF16, tag="vsb")
                    nc.vector.tensor_copy(v_sb[:], v_ps[:])
                    for j in range(4):
                        t = rb * 4 + j
                        tp_ps = ps_tp.tile([128, 128], BF16, tag="tp")
                        nc.tensor.transpose(tp_ps[:], v_sb[:, j * 128:(j + 1) * 128],
                                            ident_sb[:])
                        nc.vector.tensor_copy(va_view[:, t, 0:DH],
                                              tp_ps[:, 0:DH])
                        nc.vector.tensor_copy(va_view[:, t, DH + 1:DH + 1 + DH],
                                              tp_ps[:, DH:CPC])

                # keep PE busy across the phase transition
                brid_ps = ps1.tile([128, 512], F32, tag="brid")
                for i in range(12):
                    nc.tensor.matmul(brid_ps[:], wq_sb[:, i % KT, :],
                                     xt_sb0[:, i % KT, :],
                                     start=(i == 0), stop=(i == 11))

            # phase-3 constants load behind phase-1 traffic
            nc.scalar.dma_start(wo_sb[:], wout_d.ap().rearrange(
                "(k p) n -> p k n", p=128))
            nc.sync.dma_start(boutb_sb[:], boutb_d[:, :])
            nc.sync.dma_start(selk_sb[:],
                              selk_d.ap().rearrange("h (k p) -> h k p", p=128))
            nc.sync.dma_start(selb_sb[:], selb_d[:, :])
            nc.sync.dma_start(selbi_sb[:], selbi_d[:, :])
            # zero the shard halves each batch's A2A never writes
            zt = pp.tile([128, 512], BF16, tag="zt")
            nc.vector.memset(zt[:], 0.0)
            for b in range(B):
                for j in range(NCORES):
                    if (j // 4) != b:
                        r0 = j * SHARD_ROWS
                        nc.sync.dma_start(a2a_in[b][r0:r0 + 128, :], zt[:])
                        nc.sync.dma_start(a2a_in[b][r0 + 128:r0 + SHARD_ROWS, :],
                                          zt[0:2, :])

            # ---- Phase 2: attention, per (b, q-chunk of 512, keytile) ----
            # sc [128, 1024] packs both heads' [keys, 512 qrows] blocks side
            # by side so one exp call covers both; bufs=2 double-buffers the
            # PE-scores vs ACT-exp pipeline. PSUM: 2x2 (sc) + 2x1 (o) = 6.
            with tc.tile_pool(name="p2", bufs=2) as p2, \
                 tc.tile_pool(name="ps_sc", bufs=2, space="PSUM") as ps_sc, \
                 tc.tile_pool(name="ps_o", bufs=1, space="PSUM") as ps_o:
                QC = 512
                for b in range(B):
                    for qh in range(N // QC):
                        qbase = b * N + qh * QC
                        o_ps = [ps_o.tile([DH + 1, QC], F32, tag=f"outp{h}",
                                          name=f"ops{h}") for h in range(HPC)]
                        for kt in range(NKEYT):
                            g = b * NKEYT + kt
                            krow = b * N + kt * 128
                            sc = ps_sc.tile([128, 2 * QC], F32, tag="sc",
                                            name="sc")
                            for h in range(HPC):
                                ho = h * DH
                                nc.tensor.matmul(
                                    sc[:, h * QC:(h + 1) * QC],
                                    kt_sb[ho:ho + DH, krow:krow + 128],
                                    qt_sb[ho:ho + DH, qbase:qbase + QC],
                                    start=True, stop=True)
                            pt = p2.tile([128, 2 * QC], BF16, tag="pt",
                                         name="pt")
                            if kt % 4 == 3 and SCH_ON:
                                # Schraudolph exp on the (otherwise idle)
                                # vector engine: frees ~25% of ScalarE
                                nc.vector.tensor_scalar(
                                    out=pt[:].bitcast(I16), in0=sc[:],
                                    scalar1=SCH_SCALE,
                                    scalar2=bias2_sb[:, g:g + 1],
                                    op0=mybir.AluOpType.mult,
                                    op1=mybir.AluOpType.add)
                            else:
                                nc.scalar.activation(
                                    pt[:], sc[:],
                                    mybir.ActivationFunctionType.Exp,
                                    bias=maskb_sb[:, g:g + 1],
                                    scale=float(DH ** -0.5))
                            for h in range(HPC):
                                va_l = va_sb[:, g * VAUGW + h * (DH + 1):
                                             g * VAUGW + (h + 1) * (DH + 1)]
                                nc.tensor.matmul(
                                    o_ps[h][:], va_l, pt[:, h * QC:(h + 1) * QC],
                                    start=(kt == 0), stop=(kt == NKEYT - 1))

                        # tail: one bf16 copy + one [65, 512] DMA per head
                        # (row 64 is the softmax denominator; it travels in
                        # the A2A shard and is divided out on the receiver)
                        j = b * 4 + qh
                        for h in range(HPC):
                            onb = p2.tile([DH + 1, QC], BF16, tag=f"onb{h}",
                                          name=f"onb{h}")
                            nc.vector.tensor_copy(onb[:], o_ps[h][:])
                            r0 = j * SHARD_ROWS + h * (DH + 1)
                            nc.sync.dma_start(a2a_in[b][r0: r0 + DH + 1, :],
                                              onb[:])
                    nc.gpsimd.collective_compute(
                        "AllToAll", mybir.AluOpType.bypass,
                        replica_groups=[list(range(NCORES))],
                        ins=[a2a_in[b].ap().opt()],
                        outs=[a2a_out[b].ap().opt()])

            # ---- Phase 3: blend, normalize (selector-MM broadcast), project
            with tc.tile_pool(name="p3", bufs=1) as p3, \
                 tc.tile_pool(name="p3b", bufs=2) as p3b, \
                 tc.tile_pool(name="ps3", bufs=2, space="PSUM") as ps3, \
                 tc.tile_pool(name="ps_gr", bufs=2, space="PSUM") as ps_gr:
                # keep-warm fillers: run on PE while the last A2A is in
                # flight so the projection starts at full clock
                fil_ps = ps3.tile([128, 512], F32, tag="fil")
                for i in range(52):
                    nc.tensor.matmul(fil_ps[:, 0:256], wq_sb[:, i % KT, :],
                                     qt_sb[:, (i % KT) * 256:(i % KT) * 256 + 256],
                                     start=(i == 0), stop=(i == 51))
                ob = []
                dn = []
                for b in range(B):
                    o_t = p3.tile([128, KT, 512], BF16, tag=f"oallb{b}",
                                  name=f"oallb{b}")
                    d_t = p3.tile([2 * NCORES, 512], BF16, tag=f"denb{b}",
                                  name=f"denb{b}")
                    av = a2a_out[b].ap().rearrange("(j q) n -> q j n",
                                                   q=SHARD_ROWS)
                    nc.sync.dma_start(d_t[0:NCORES, :], av[DH:DH + 1, :, :])
                    nc.sync.dma_start(d_t[NCORES:2 * NCORES, :],
                                      av[CPC + 1:CPC + 2, :, :])
                    nc.sync.dma_start(o_t[0:DH, :, :], av[0:DH, :, :])
                    nc.scalar.dma_start(o_t[DH:CPC, :, :], av[DH + 1:CPC + 1, :, :])
                    ob.append(o_t)
                    dn.append(d_t)
                # blend mine = b0*sel + b1*(1-sel)
                oall_sb = p3.tile([128, KT, 512], BF16, tag="oall")
                t1_sb = p3.tile([128, KT, 512], BF16, tag="t1")
                nc.vector.tensor_scalar_mul(oall_sb[:], ob[0][:], selb_sb[:])
                nc.vector.tensor_scalar_mul(t1_sb[:], ob[1][:], selbi_sb[:])
                nc.vector.tensor_add(oall_sb[:], oall_sb[:], t1_sb[:])
                den_sb = p3.tile([2 * NCORES, 512], F32, tag="den")
                dt1_sb = p3.tile([2 * NCORES, 512], F32, tag="dt1")
                nc.vector.tensor_scalar_mul(den_sb[:], dn[0][:],
                                            selb_sb[0:2 * NCORES, :])
                nc.vector.tensor_scalar_mul(dt1_sb[:], dn[1][:],
                                            selbi_sb[0:2 * NCORES, :])
                nc.vector.tensor_add(den_sb[:], den_sb[:], dt1_sb[:])
                # den rows: head of chan p in peer-block kt is row
                # (p//64)*8 + kt; spread recips across partitions via
                # selector matmuls
                recip_sb = p3.tile([2 * NCORES, 512], F32, tag="recip")
                nc.vector.reciprocal_approx_fast(recip_sb[:], den_sb[:])
                recb_sb = p3.tile([2 * NCORES, 512], BF16, tag="recb")
                nc.vector.tensor_copy(recb_sb[:], recip_sb[:])

                onorm_sb = p3.tile([128, KT, 512], BF16, tag="onorm")
                for kt in range(KT):
                    gr_ps = ps_gr.tile([128, 512], F32, tag="gr")
                    nc.tensor.matmul(gr_ps[:], selk_sb[:, kt, :], recb_sb[:],
                                     start=True, stop=True)
                    nc.vector.tensor_mul(onorm_sb[:, kt, :], oall_sb[:, kt, :],
                                         gr_ps[:])

                for rw in range(4):
                    y_ps = ps3.tile([128, DM], F32, tag="y")
                    for kt in range(KT):
                        st, sp = kt == 0, kt == KT - 1
                        for nb in range(2):
                            nc.tensor.matmul(
                                y_ps[:, nb * 512:(nb + 1) * 512],
                                onorm_sb[:, kt, rw * 128:(rw + 1) * 128],
                                wo_sb[:, kt, nb * 512:(nb + 1) * 512],
                                start=st, stop=sp)
                    y_sb = p3b.tile([128, DM], F32, tag="y_sb")
                    nc.vector.tensor_add(y_sb[:], y_ps[:], boutb_sb[:])
                    eng = nc.sync if rw % 2 == 0 else nc.scalar
                    eng.dma_start(out_d[rw * 128:(rw + 1) * 128, :], y_sb[:])

    nc.compile()
    return nc


_NC_CACHE = None


def kernel(x, mask, pos_emb, Wq, Wkv, Wout, bout):
    global LAST_EXEC_TIME_NS, LAST_TRACE_DIR, _NC_CACHE

    x = np.asarray(x, dtype=np.float32)
    mask = np.asarray(mask)
    pos_emb = np.asarray(pos_emb, dtype=np.float32)
    Wq = np.asarray(Wq, dtype=np.float32)
    Wkv = np.asarray(Wkv, dtype=np.float32)
    Wout = np.asarray(Wout, dtype=np.float32)
    bout = np.asarray(bout, dtype=np.float32)

    bf = ml_dtypes.bfloat16
    xt = np.ascontiguousarray(x.reshape(R, DM).T).astype(bf)
    wk_full = Wkv[:, :H * DH]
    wv_full = Wkv[:, H * DH:]
    cost = np.ascontiguousarray(np.tile(np.cos(pos_emb).T, (HPC, 1))).astype(bf)
    sint = np.ascontiguousarray(np.tile(np.sin(pos_emb).T, (HPC, 1))).astype(bf)
    # -30 (not -1e5) so the Schraudolph int16 path stays unsaturated;
    # exp(-30) ~ 1e-13 is dead weight either way
    maskb = np.ascontiguousarray(
        np.where(mask.reshape(R), 0.0, -30.0).astype(np.float32)
        .reshape(R // 128, 128).T)
    boutb = np.ascontiguousarray(
        np.broadcast_to(bout[None, :], (128, DM)).astype(np.float32))
    wqr = _rot_cols(Wq)
    wkr = _rot_cols(wk_full)
    ident = np.eye(128, dtype=bf)
    # selk[row, kt*128 + p] = 1 iff den-row `row` covers partition p of the
    # kt-th chan block: inner chan kt*128+p is peer j=kt, local head p//64;
    # its den row in d_t is (p//64)*8 + kt.
    selk = np.zeros((16, KT * 128), dtype=bf)
    for ktb in range(KT):
        for p in range(128):
            selk[(p // 64) * 8 + ktb, ktb * 128 + p] = 1.0
    in_maps = []
    for c in range(NCORES):
        cols = slice(c * CPC, (c + 1) * CPC)
        in_maps.append({
            "xt": xt,
            "wq": np.ascontiguousarray(Wq[:, cols]).astype(bf),
            "wqr": np.ascontiguousarray(wqr[:, cols]).astype(bf),
            "wk": np.ascontiguousarray(wk_full[:, cols]).astype(bf),
            "wkr": np.ascontiguousarray(wkr[:, cols]).astype(bf),
            "wv": np.ascontiguousarray(wv_full[:, cols]).astype(bf),
            "wout": Wout.astype(bf),
            "boutb": boutb,
            "cost": cost,
            "sint": sint,
            "maskb": maskb,
            "ident": ident,
            "selk": selk,
            "selb": np.full((128, 1), 1.0 if c < 4 else 0.0, dtype=np.float32),
            "selbi": np.full((128, 1), 0.0 if c < 4 else 1.0, dtype=np.float32),
        })

    dbg = bool(int(os.environ.get("BASS_KERNEL_DEBUG", "0")))
    if _NC_CACHE is None:
        _NC_CACHE = build(dbg=dbg)
    nc = _NC_CACHE

    trace = bool(int(os.environ.get("BASS_KERNEL_TRACE", "0")))
    kwargs = {}
    if trace:
        _install_trace_shim()
        tdir = os.environ.get("BASS_TRACE_DIR", "/tmp/bass_trace_out")
        os.makedirs(tdir, exist_ok=True)
        kwargs["tmpdir"] = tdir
    res = bass_utils.run_bass_kernel_spmd(
        nc, in_maps, core_ids=list(range(NCORES)), trace=trace, **kwargs)
    LAST_EXEC_TIME_NS = res.exec_time_ns
    if res.instructions_and_trace is not None:
        LAST_TRACE_DIR = res.instructions_and_trace[1]
        globals()["LAST_INSTS"] = res.instructions_and_trace[0]

    globals()["LAST_RESULTS"] = res.results
    y = np.concatenate([res.results[c]["out"] for c in range(NCORES)], axis=0)
    return y.reshape(B, N, DM)
